# revision 33
# baseline (speedup 1.0000x reference)
"""Trainium2 Bass kernel for nn_ClusterLookup (vq_codebook).

reference math:
    cn = l2norm(clusters, axis=1)                 # [N, C]
    xn = l2norm(x, axis=1)                        # [B, C, H, W]
    inner = einsum('bchw,nc->bnhw', xn, cn)       # cosine sims
    probs = softmax(2 * inner, axis=1)
    loss  = -mean(sum(probs * inner, axis=1))
    return loss, probs

Strategy (8 cores, data-parallel over batch, 4 batches/core):
  - x and the (host-normalized) codebook are cast to bf16 on the host: the
    512-term contractions average out per-element rounding (~1e-4 abs on the
    logits), halve HBM traffic, enable single-pass matmuls + fast weight
    load (fp32 matmuls are double-pumped on TRN2).
  - pixels on PSUM partitions, clusters on the free dim:
      raw[pix, n] = sum_c x[c, pix] * cn[c, n]   (4 K-chunk matmuls)
      ss[pix]     = sum_c x2[c, pix]             (x2 = x*x on DVE, ones-matmul)
  - rs2 = 2/sqrt(ss) computed as Exp(-0.5*Ln(ss) + ln2): keeps every ACT op
    in the single `natural_log_exp_and_others` table set.
  - t = raw * rs2 (= 2*cos, broadcast multiply); e = Exp(t); softmax along
    the free dim (27).  Loss per pixel = rs2 * sum_n probs*raw, finished on
    the host: loss = -sum/(2*B*HW).
  - probs transposed on TensorE ([pix, T*27] -> [T*27, pix]) so the HBM
    store is contiguous 512B runs; host only reshapes.
"""

import sys

if "/opt/trn_rl_repo" not in sys.path:
    sys.path.insert(0, "/opt/trn_rl_repo")

import math

import numpy as np

B, C, H, W = 32, 512, 56, 56
N = 27
HW = H * W                    # 3136
P = 128
NCORES = 8
BPC = B // NCORES             # 4 batches per core
NCHUNK = C // P               # 4 contraction chunks
FULL_TILES = HW // P          # 24 full 128-pixel tiles per batch
REM = HW - FULL_TILES * P     # 64 remainder pixels
GT = 4                        # pixel subtiles per group
NG_FULL = FULL_TILES // GT    # 6 full groups per batch
NSUB_B = FULL_TILES + 1       # 25 subtile slots per batch (loss columns)
NSUB = BPC * NSUB_B           # 100 loss columns per core

# (pixel offset, valid pixels per subtile, subtiles in group)
GROUPS = [(g * GT * P, P, GT) for g in range(NG_FULL)] + [(FULL_TILES * P, REM, 1)]

_NC = None
LAST_RESULT = None
TRACE_TMPDIR = None  # set by test harness to keep trace artifacts


def _build_nc():
    import concourse.bacc as bacc
    import concourse.bass as bass
    import concourse.tile as tile
    from concourse import mybir
    from concourse.masks import make_identity

    f32 = mybir.dt.float32
    bf16 = mybir.dt.bfloat16
    AF = mybir.ActivationFunctionType

    nc = bacc.Bacc()
    x_in = nc.dram_tensor("x_in", [BPC, C, HW], bf16, kind="ExternalInput")
    cl_in = nc.dram_tensor("cl_in", [C, N], bf16, kind="ExternalInput")
    probs_out = nc.dram_tensor("probs_out", [BPC, N, HW], f32, kind="ExternalOutput")
    loss_out = nc.dram_tensor("loss_out", [P, NSUB], f32, kind="ExternalOutput")

    def bcast(ap, n_inner):
        """[p, T] per-pixel scalars -> [p, T, n_inner] step-0 broadcast AP."""
        return bass.AP(
            tensor=ap.tensor,
            offset=ap.offset,
            ap=[*ap.ap, [0, n_inner]],
        )

    with tile.TileContext(nc) as tc:
        with (
            tc.tile_pool(name="singles", bufs=1) as singles,
            tc.tile_pool(name="xp", bufs=3) as xp,
            tc.tile_pool(name="x2p", bufs=2) as x2p,
            tc.tile_pool(name="bt", bufs=2) as bt,
            tc.tile_pool(name="smx", bufs=3) as smx,
            tc.tile_pool(name="outp", bufs=4) as outp,
            tc.tile_pool(name="pp", bufs=2, space="PSUM") as pp,
        ):
            cl_sb = singles.tile([P, NCHUNK, N], bf16)
            nc.sync.dma_start(out=cl_sb, in_=cl_in.rearrange("(k p) n -> p k n", p=P))
            ident = singles.tile([P, P], f32)
            make_identity(nc, ident)
            ones_bf = singles.tile([P, 1], bf16)
            nc.vector.memset(ones_bf, 1.0)
            loss_sb = singles.tile([P, NSUB], f32)
            nc.vector.memset(loss_sb, 0.0)
            ln2_sb = singles.tile([P, 1], f32)
            nc.vector.memset(ln2_sb, float(math.log(2.0)))

            # PE-clock absorbers: pull cross-engine ticks into PE's vector
            # clock so real matmuls rarely need multi-wait splitting.
            dummy_ps = pp.tile([P, N], f32, tag="dummy", bufs=1)
            nc.tensor.matmul(dummy_ps[:N, :N], lhsT=cl_sb[:, 0, :], rhs=cl_sb[:, 0, :])
            nc.tensor.matmul(dummy_ps[:1, :1], lhsT=ones_bf, rhs=ones_bf)

            def do_batch(ib):
                x_sb = xp.tile([P, NCHUNK, HW], bf16, tag="x")
                nc.sync.dma_start(
                    out=x_sb, in_=x_in[ib].rearrange("(k p) q -> p k q", p=P)
                )
                x2_sb = x2p.tile([P, NCHUNK, HW], bf16, tag="x2")
                # split the squares across DVE/DVE/ACT/GPSIMD to balance
                nc.vector.tensor_mul(out=x2_sb[:, 0], in0=x_sb[:, 0], in1=x_sb[:, 0])
                nc.vector.tensor_mul(out=x2_sb[:, 1], in0=x_sb[:, 1], in1=x_sb[:, 1])
                nc.scalar.activation(x2_sb[:, 2], x_sb[:, 2], AF.Square)
                nc.gpsimd.tensor_mul(out=x2_sb[:, 3], in0=x_sb[:, 3], in1=x_sb[:, 3])
                # absorb this batch's x DMA + first square into the PE clock
                nc.tensor.matmul(
                    dummy_ps[:1, :N], lhsT=x_sb[:, 0, 0:1], rhs=cl_sb[:, 0, :]
                )
                nc.tensor.matmul(
                    dummy_ps[:1, :1], lhsT=x2_sb[:, 0, 0:1], rhs=ones_bf
                )

                rs2_b = bt.tile([P, NSUB_B], f32, tag="rs2")
                qe_b = bt.tile([P, NSUB_B], f32, tag="qe")
                r_b = bt.tile([P, NSUB_B], f32, tag="r")

                for gi, (off, m, T) in enumerate(GROUPS):
                    cols = T * N
                    gc = gi * GT
                    raw_ps = pp.tile([P, GT * N], f32, tag="raw_ps", bufs=3)
                    ss_ps = pp.tile([P, GT], f32, tag="ss_ps")
                    # ss matmuls first: the norm chain (ln/rs2 on ACT) then
                    # hides under the main matmuls that follow
                    for j in range(T):
                        o = off + j * P
                        for k in range(NCHUNK):
                            nc.tensor.matmul(
                                ss_ps[:m, j:j + 1],
                                lhsT=x2_sb[:, k, o:o + m],
                                rhs=ones_bf,
                                start=(k == 0),
                                stop=(k == NCHUNK - 1),
                            )
                    ln_g = smx.tile([P, GT], f32, tag="ln")
                    nc.scalar.activation(ln_g[:m, :T], ss_ps[:m, :T], AF.Ln)
                    nc.scalar.activation(
                        rs2_b[:m, gc:gc + T], ln_g[:m, :T], AF.Exp,
                        bias=ln2_sb[:m], scale=-0.5,
                    )
                    for j in range(T):
                        o = off + j * P
                        for k in range(NCHUNK):
                            nc.tensor.matmul(
                                raw_ps[:m, j * N:(j + 1) * N],
                                lhsT=x_sb[:, k, o:o + m],
                                rhs=cl_sb[:, k, :],
                                start=(k == 0),
                                stop=(k == NCHUNK - 1),
                            )
                    # t = raw * rs2  (= 2*cos)
                    t_sb = smx.tile([P, GT * N], f32, tag="t")
                    nc.vector.tensor_mul(
                        out=t_sb[:m, :cols].rearrange("p (t n) -> p t n", n=N),
                        in0=raw_ps[:m, :cols].rearrange("p (t n) -> p t n", n=N),
                        in1=bcast(rs2_b[:m, gc:gc + T], N),
                    )
                    e_sb = smx.tile([P, GT * N], f32, tag="e")
                    nc.scalar.activation(e_sb[:m, :cols], t_sb[:m, :cols], AF.Exp)
                    # w = e*raw right away: frees raw_ps a whole chain earlier
                    w_sb = smx.tile([P, GT * N], f32, tag="w")
                    nc.vector.tensor_mul(
                        out=w_sb[:m, :cols],
                        in0=e_sb[:m, :cols],
                        in1=raw_ps[:m, :cols],
                    )
                    nc.vector.reduce_sum(
                        out=qe_b[:m, gc:gc + T],
                        in_=w_sb[:m, :cols].rearrange("p (t n) -> p t n", n=N),
                        axis=mybir.AxisListType.X,
                    )
                    s_e = smx.tile([P, GT], f32, tag="s_e")
                    nc.vector.reduce_sum(
                        out=s_e[:m, :T],
                        in_=e_sb[:m, :cols].rearrange("p (t n) -> p t n", n=N),
                        axis=mybir.AxisListType.X,
                    )
                    nc.vector.reciprocal(out=r_b[:m, gc:gc + T], in_=s_e[:m, :T])
                    probs_sb = smx.tile([P, GT * N], f32, tag="probs")
                    nc.vector.tensor_mul(
                        out=probs_sb[:m, :cols].rearrange("p (t n) -> p t n", n=N),
                        in0=e_sb[:m, :cols].rearrange("p (t n) -> p t n", n=N),
                        in1=bcast(r_b[:m, gc:gc + T], N),
                    )
                    # transpose probs for a contiguous HBM store
                    tp_ps = pp.tile([GT * N, P], f32, tag="tp")
                    nc.tensor.transpose(
                        tp_ps[:cols, :m], probs_sb[:m, :cols], ident[:m, :m]
                    )
                    out_sb = outp.tile([GT * N, P], f32, tag="out")
                    nc.scalar.copy(out=out_sb[:cols, :m], in_=tp_ps[:cols, :m])
                    if T == GT:
                        dst = probs_out[ib, :, off:off + T * P].rearrange(
                            "n (j p) -> j n p", p=P
                        )
                        nc.sync.dma_start(out=dst, in_=out_sb[:cols, :])
                    else:
                        nc.sync.dma_start(
                            out=probs_out[ib, :, off:off + m], in_=out_sb[:N, :m]
                        )
                # loss tail: per-pixel sum_n probs*t = (sum_n e*raw) * r * rs2
                # (split: the remainder column only has REM valid rows)
                full = NSUB_B - 1
                lp = bt.tile([P, NSUB_B], f32, tag="lp")
                nc.vector.tensor_mul(
                    out=lp[:, :full], in0=qe_b[:, :full], in1=r_b[:, :full]
                )
                nc.vector.tensor_mul(
                    out=loss_sb[:, ib * NSUB_B:ib * NSUB_B + full],
                    in0=lp[:, :full],
                    in1=rs2_b[:, :full],
                )
                nc.vector.tensor_mul(
                    out=lp[:REM, full:], in0=qe_b[:REM, full:], in1=r_b[:REM, full:]
                )
                nc.vector.tensor_mul(
                    out=loss_sb[:REM, ib * NSUB_B + full:(ib + 1) * NSUB_B],
                    in0=lp[:REM, full:],
                    in1=rs2_b[:REM, full:],
                )

            for ib in range(BPC):
                do_batch(ib)

            nc.sync.dma_start(out=loss_out[:, :], in_=loss_sb)

    nc.finalize()  # Bacc: wait splitting, reg alloc, ACT table loads

    # The table-load inserter picks the FIRST set containing each function,
    # so Ln -> natural_log and Exp -> exp_and_others thrash the table RAM
    # (~1.3us per reload, every group). All our functions (Ln, Exp, Copy,
    # Square) live together in `natural_log_exp_and_others`; retarget every
    # load there and drop the now-redundant reloads.
    from concourse.hw_specs import get_activation_tables

    tables = list(get_activation_tables(nc.m.arch).keys())
    target = tables.index("natural_log_exp_and_others")
    for bb in nc.main_func.blocks:
        keep = []
        seen = False
        for inst in bb.instructions:
            if isinstance(inst, mybir.InstLoadActFuncSet):
                inst.act_func_set_id = target
                has_sync = bool(
                    inst.sync_info
                    and (inst.sync_info.on_wait or inst.sync_info.on_update)
                )
                if seen and not has_sync:
                    continue
                seen = True
            keep.append(inst)
        if len(keep) != len(bb.instructions):
            bb.instructions[:] = keep
    return nc


def _get_nc():
    global _NC
    if _NC is None:
        _NC = _build_nc()
    return _NC


def host_prep_clusters(clusters):
    import ml_dtypes

    cl = np.asarray(clusters, dtype=np.float32)
    nrm = np.sqrt((cl * cl).sum(axis=1, keepdims=True, dtype=np.float32))
    cln = cl / np.maximum(nrm, np.float32(1e-12))
    return np.ascontiguousarray(cln.T).astype(ml_dtypes.bfloat16)  # [C, N]


def make_in_maps(x, clusters):
    import ml_dtypes

    x = np.asarray(x)
    clT = host_prep_clusters(clusters)
    xs = x.reshape(NCORES, BPC, C, HW).astype(ml_dtypes.bfloat16)
    return [
        {"x_in": np.ascontiguousarray(xs[i]), "cl_in": clT} for i in range(NCORES)
    ]


def kernel(x, clusters):
    global LAST_RESULT
    from concourse import bass_utils

    in_maps = make_in_maps(x, clusters)
    nc = _get_nc()
    res = bass_utils.run_bass_kernel_spmd(
        nc, in_maps, core_ids=list(range(NCORES)), tmpdir=TRACE_TMPDIR
    )
    LAST_RESULT = res
    probs = np.concatenate([r["probs_out"] for r in res.results], axis=0)
    probs = np.ascontiguousarray(probs.reshape(B, N, H, W), dtype=np.float32)
    tot = 0.0
    for r in res.results:
        tot += float(r["loss_out"].astype(np.float64).sum())
    loss = -(tot / (2.0 * B * HW))
    return np.float32(loss), probs


# revision 36
# speedup vs baseline: 1.0291x; 1.0291x over previous
"""Trainium2 Bass kernel for nn_ClusterLookup (vq_codebook).

reference math:
    cn = l2norm(clusters, axis=1)                 # [N, C]
    xn = l2norm(x, axis=1)                        # [B, C, H, W]
    inner = einsum('bchw,nc->bnhw', xn, cn)       # cosine sims
    probs = softmax(2 * inner, axis=1)
    loss  = -mean(sum(probs * inner, axis=1))
    return loss, probs

Strategy (8 cores, data-parallel over batch, 4 batches/core):
  - x and the (host-normalized) codebook are cast to bf16 on the host: the
    512-term contractions average out per-element rounding (~1e-4 abs on the
    logits), halve HBM traffic, enable single-pass matmuls + fast weight
    load (fp32 matmuls are double-pumped on TRN2).
  - pixels on PSUM partitions, clusters on the free dim:
      raw[pix, n] = sum_c x[c, pix] * cn[c, n]   (4 K-chunk matmuls)
      ss[pix]     = sum_c x2[c, pix]             (x2 = x*x on DVE, ones-matmul)
  - rs2 = 2/sqrt(ss) computed as Exp(-0.5*Ln(ss) + ln2): keeps every ACT op
    in the single `natural_log_exp_and_others` table set.
  - t = raw * rs2 (= 2*cos, broadcast multiply); e = Exp(t); softmax along
    the free dim (27).  Loss per pixel = rs2 * sum_n probs*raw, finished on
    the host: loss = -sum/(2*B*HW).
  - probs transposed on TensorE ([pix, T*27] -> [T*27, pix]) so the HBM
    store is contiguous 512B runs; host only reshapes.
"""

import sys

if "/opt/trn_rl_repo" not in sys.path:
    sys.path.insert(0, "/opt/trn_rl_repo")

import math

import numpy as np

B, C, H, W = 32, 512, 56, 56
N = 27
HW = H * W                    # 3136
P = 128
NCORES = 8
BPC = B // NCORES             # 4 batches per core
NCHUNK = C // P               # 4 contraction chunks
FULL_TILES = HW // P          # 24 full 128-pixel tiles per batch
REM = HW - FULL_TILES * P     # 64 remainder pixels
GT = 4                        # pixel subtiles per group
NG_FULL = FULL_TILES // GT    # 6 full groups per batch
NSUB_B = FULL_TILES + 1       # 25 subtile slots per batch (loss columns)
NSUB = BPC * NSUB_B           # 100 loss columns per core

# (pixel offset, valid pixels per subtile, subtiles in group)
GROUPS = [(g * GT * P, P, GT) for g in range(NG_FULL)] + [(FULL_TILES * P, REM, 1)]

_NC = None
LAST_RESULT = None
TRACE_TMPDIR = None  # set by test harness to keep trace artifacts


def _build_nc():
    import concourse.bacc as bacc
    import concourse.bass as bass
    import concourse.tile as tile
    from concourse import mybir
    from concourse.masks import make_identity

    f32 = mybir.dt.float32
    bf16 = mybir.dt.bfloat16
    AF = mybir.ActivationFunctionType

    nc = bacc.Bacc()
    x_in = nc.dram_tensor("x_in", [BPC, C, HW], bf16, kind="ExternalInput")
    cl_in = nc.dram_tensor("cl_in", [C, N], bf16, kind="ExternalInput")
    probs_out = nc.dram_tensor("probs_out", [BPC, N, HW], f32, kind="ExternalOutput")
    loss_out = nc.dram_tensor("loss_out", [P, NSUB], f32, kind="ExternalOutput")

    def bcast(ap, n_inner):
        """[p, T] per-pixel scalars -> [p, T, n_inner] step-0 broadcast AP."""
        return bass.AP(
            tensor=ap.tensor,
            offset=ap.offset,
            ap=[*ap.ap, [0, n_inner]],
        )

    with tile.TileContext(nc) as tc:
        with (
            tc.tile_pool(name="singles", bufs=1) as singles,
            tc.tile_pool(name="xp", bufs=3) as xp,
            tc.tile_pool(name="x2p", bufs=2) as x2p,
            tc.tile_pool(name="bt", bufs=2) as bt,
            tc.tile_pool(name="smx", bufs=3) as smx,
            tc.tile_pool(name="outp", bufs=4) as outp,
            tc.tile_pool(name="pp", bufs=2, space="PSUM") as pp,
        ):
            cl_sb = singles.tile([P, NCHUNK, N], bf16)
            nc.sync.dma_start(out=cl_sb, in_=cl_in.rearrange("(k p) n -> p k n", p=P))
            ident = singles.tile([P, P], f32)
            make_identity(nc, ident)
            ones_bf = singles.tile([P, 1], bf16)
            nc.vector.memset(ones_bf, 1.0)
            loss_sb = singles.tile([P, NSUB], f32)
            nc.vector.memset(loss_sb, 0.0)
            ln2_sb = singles.tile([P, 1], f32)
            nc.vector.memset(ln2_sb, float(math.log(2.0)))

            # PE-clock absorbers: pull cross-engine ticks into PE's vector
            # clock so real matmuls rarely need multi-wait splitting.
            dummy_ps = pp.tile([P, N], f32, tag="dummy", bufs=1)
            nc.tensor.matmul(dummy_ps[:N, :N], lhsT=cl_sb[:, 0, :], rhs=cl_sb[:, 0, :])
            nc.tensor.matmul(dummy_ps[:1, :1], lhsT=ones_bf, rhs=ones_bf)

            # Issue every x load up-front on the otherwise-empty sync queue:
            # triggers gate only on their pool slot (bufs=3), so three batches
            # stream in while compute runs, with no probs-out HOL blocking.
            x_tiles = []
            for ib in range(BPC):
                x_sb = xp.tile([P, NCHUNK, HW], bf16, tag="x", name=f"x_sb{ib}")
                nc.sync.dma_start(
                    out=x_sb, in_=x_in[ib].rearrange("(k p) q -> p k q", p=P)
                )
                x_tiles.append(x_sb)

            def do_batch(ib):
                x_sb = x_tiles[ib]
                x2_sb = x2p.tile([P, NCHUNK, HW], bf16, tag="x2")
                # split the squares across DVE/DVE/ACT/GPSIMD to balance
                nc.vector.tensor_mul(out=x2_sb[:, 0], in0=x_sb[:, 0], in1=x_sb[:, 0])
                nc.vector.tensor_mul(out=x2_sb[:, 1], in0=x_sb[:, 1], in1=x_sb[:, 1])
                nc.scalar.activation(x2_sb[:, 2], x_sb[:, 2], AF.Square)
                nc.gpsimd.tensor_mul(out=x2_sb[:, 3], in0=x_sb[:, 3], in1=x_sb[:, 3])
                # absorb this batch's x DMA + first square into the PE clock
                nc.tensor.matmul(
                    dummy_ps[:1, :N], lhsT=x_sb[:, 0, 0:1], rhs=cl_sb[:, 0, :]
                )
                nc.tensor.matmul(
                    dummy_ps[:1, :1], lhsT=x2_sb[:, 0, 0:1], rhs=ones_bf
                )

                rs2_b = bt.tile([P, NSUB_B], f32, tag="rs2")
                qe_b = bt.tile([P, NSUB_B], f32, tag="qe")
                r_b = bt.tile([P, NSUB_B], f32, tag="r")

                for gi, (off, m, T) in enumerate(GROUPS):
                    cols = T * N
                    gc = gi * GT
                    raw_ps = pp.tile([P, GT * N], f32, tag="raw_ps", bufs=3)
                    ss_ps = pp.tile([P, GT], f32, tag="ss_ps")
                    # ss matmuls first: the norm chain (ln/rs2 on ACT) then
                    # hides under the main matmuls that follow
                    for j in range(T):
                        o = off + j * P
                        for k in range(NCHUNK):
                            nc.tensor.matmul(
                                ss_ps[:m, j:j + 1],
                                lhsT=x2_sb[:, k, o:o + m],
                                rhs=ones_bf,
                                start=(k == 0),
                                stop=(k == NCHUNK - 1),
                            )
                    ln_g = smx.tile([P, GT], f32, tag="ln")
                    nc.scalar.activation(ln_g[:m, :T], ss_ps[:m, :T], AF.Ln)
                    nc.scalar.activation(
                        rs2_b[:m, gc:gc + T], ln_g[:m, :T], AF.Exp,
                        bias=ln2_sb[:m], scale=-0.5,
                    )
                    for j in range(T):
                        o = off + j * P
                        for k in range(NCHUNK):
                            nc.tensor.matmul(
                                raw_ps[:m, j * N:(j + 1) * N],
                                lhsT=x_sb[:, k, o:o + m],
                                rhs=cl_sb[:, k, :],
                                start=(k == 0),
                                stop=(k == NCHUNK - 1),
                            )
                    # t = raw * rs2  (= 2*cos)
                    t_sb = smx.tile([P, GT * N], f32, tag="t")
                    nc.vector.tensor_mul(
                        out=t_sb[:m, :cols].rearrange("p (t n) -> p t n", n=N),
                        in0=raw_ps[:m, :cols].rearrange("p (t n) -> p t n", n=N),
                        in1=bcast(rs2_b[:m, gc:gc + T], N),
                    )
                    e_sb = smx.tile([P, GT * N], f32, tag="e")
                    nc.scalar.activation(e_sb[:m, :cols], t_sb[:m, :cols], AF.Exp)
                    # w = e*raw right away: frees raw_ps a whole chain earlier
                    w_sb = smx.tile([P, GT * N], f32, tag="w")
                    nc.vector.tensor_mul(
                        out=w_sb[:m, :cols],
                        in0=e_sb[:m, :cols],
                        in1=raw_ps[:m, :cols],
                    )
                    nc.vector.reduce_sum(
                        out=qe_b[:m, gc:gc + T],
                        in_=w_sb[:m, :cols].rearrange("p (t n) -> p t n", n=N),
                        axis=mybir.AxisListType.X,
                    )
                    s_e = smx.tile([P, GT], f32, tag="s_e")
                    nc.vector.reduce_sum(
                        out=s_e[:m, :T],
                        in_=e_sb[:m, :cols].rearrange("p (t n) -> p t n", n=N),
                        axis=mybir.AxisListType.X,
                    )
                    nc.vector.reciprocal(out=r_b[:m, gc:gc + T], in_=s_e[:m, :T])
                    probs_sb = smx.tile([P, GT * N], f32, tag="probs")
                    nc.vector.tensor_mul(
                        out=probs_sb[:m, :cols].rearrange("p (t n) -> p t n", n=N),
                        in0=e_sb[:m, :cols].rearrange("p (t n) -> p t n", n=N),
                        in1=bcast(r_b[:m, gc:gc + T], N),
                    )
                    # transpose probs for a contiguous HBM store
                    tp_ps = pp.tile([GT * N, P], f32, tag="tp")
                    nc.tensor.transpose(
                        tp_ps[:cols, :m], probs_sb[:m, :cols], ident[:m, :m]
                    )
                    out_sb = outp.tile([GT * N, P], f32, tag="out")
                    nc.scalar.copy(out=out_sb[:cols, :m], in_=tp_ps[:cols, :m])
                    if T == GT:
                        dst = probs_out[ib, :, off:off + T * P].rearrange(
                            "n (j p) -> j n p", p=P
                        )
                        nc.gpsimd.dma_start(out=dst, in_=out_sb[:cols, :])
                    else:
                        nc.gpsimd.dma_start(
                            out=probs_out[ib, :, off:off + m], in_=out_sb[:N, :m]
                        )
                # loss tail: per-pixel sum_n probs*t = (sum_n e*raw) * r * rs2
                # (split: the remainder column only has REM valid rows)
                full = NSUB_B - 1
                lp = bt.tile([P, NSUB_B], f32, tag="lp")
                nc.vector.tensor_mul(
                    out=lp[:, :full], in0=qe_b[:, :full], in1=r_b[:, :full]
                )
                nc.vector.tensor_mul(
                    out=loss_sb[:, ib * NSUB_B:ib * NSUB_B + full],
                    in0=lp[:, :full],
                    in1=rs2_b[:, :full],
                )
                nc.vector.tensor_mul(
                    out=lp[:REM, full:], in0=qe_b[:REM, full:], in1=r_b[:REM, full:]
                )
                nc.vector.tensor_mul(
                    out=loss_sb[:REM, ib * NSUB_B + full:(ib + 1) * NSUB_B],
                    in0=lp[:REM, full:],
                    in1=rs2_b[:REM, full:],
                )

            for ib in range(BPC):
                do_batch(ib)

            nc.gpsimd.dma_start(out=loss_out[:, :], in_=loss_sb)

    nc.finalize()  # Bacc: wait splitting, reg alloc, ACT table loads

    # The table-load inserter picks the FIRST set containing each function,
    # so Ln -> natural_log and Exp -> exp_and_others thrash the table RAM
    # (~1.3us per reload, every group). All our functions (Ln, Exp, Copy,
    # Square) live together in `natural_log_exp_and_others`; retarget every
    # load there and drop the now-redundant reloads.
    from concourse.hw_specs import get_activation_tables

    tables = list(get_activation_tables(nc.m.arch).keys())
    target = tables.index("natural_log_exp_and_others")
    for bb in nc.main_func.blocks:
        keep = []
        seen = False
        for inst in bb.instructions:
            if isinstance(inst, mybir.InstLoadActFuncSet):
                inst.act_func_set_id = target
                has_sync = bool(
                    inst.sync_info
                    and (inst.sync_info.on_wait or inst.sync_info.on_update)
                )
                if seen and not has_sync:
                    continue
                seen = True
            keep.append(inst)
        if len(keep) != len(bb.instructions):
            bb.instructions[:] = keep
    return nc


def _get_nc():
    global _NC
    if _NC is None:
        _NC = _build_nc()
    return _NC


def host_prep_clusters(clusters):
    import ml_dtypes

    cl = np.asarray(clusters, dtype=np.float32)
    nrm = np.sqrt((cl * cl).sum(axis=1, keepdims=True, dtype=np.float32))
    cln = cl / np.maximum(nrm, np.float32(1e-12))
    return np.ascontiguousarray(cln.T).astype(ml_dtypes.bfloat16)  # [C, N]


def make_in_maps(x, clusters):
    import ml_dtypes

    x = np.asarray(x)
    clT = host_prep_clusters(clusters)
    xs = x.reshape(NCORES, BPC, C, HW).astype(ml_dtypes.bfloat16)
    return [
        {"x_in": np.ascontiguousarray(xs[i]), "cl_in": clT} for i in range(NCORES)
    ]


def kernel(x, clusters):
    global LAST_RESULT
    from concourse import bass_utils

    in_maps = make_in_maps(x, clusters)
    nc = _get_nc()
    res = bass_utils.run_bass_kernel_spmd(
        nc, in_maps, core_ids=list(range(NCORES)), tmpdir=TRACE_TMPDIR
    )
    LAST_RESULT = res
    probs = np.concatenate([r["probs_out"] for r in res.results], axis=0)
    probs = np.ascontiguousarray(probs.reshape(B, N, H, W), dtype=np.float32)
    tot = 0.0
    for r in res.results:
        tot += float(r["loss_out"].astype(np.float64).sum())
    loss = -(tot / (2.0 * B * HW))
    return np.float32(loss), probs
